# revision 47
# baseline (speedup 1.0000x reference)
"""TAGConv GNN classifier on 8 Trainium2 NeuronCores.

Sharding: nodes split into 8 contiguous slices (6250/core, padded to 6272);
edges live on the core that owns their dst. Each hop: every core gathers
src rows from a replicated norm-prescaled bf16 node table in HBM
(dma_gather, int16 indices -> split-table trick), segment-sums them into
its dst slice with one-hot matmuls on TensorE (PSUM accumulation), rescales
by norm, and all-gathers its slice of the next table. Readout partial sums
per graph are all-reduced, then every core computes the (identical) logits.

Per-group work is uniform (chunk counts padded to the max) so each pass is
a single hardware For_i loop over the 49 dst groups -> ~15x fewer
instructions than full unrolling, which cuts per-call program
serialize/load overhead. All inputs ship as ONE packed int8 tensor per
core (x quantized to int5 with per-node scales, bf16 weights,
de-replicated int16 gather indices, uint8 slots, fp32 misc) to cut
host->device transfer bytes ~37x vs naive fp32 uploads; a persistent XLA
compilation cache removes the per-call PJRT recompile.
"""
import os
import tempfile

import numpy as np
import ml_dtypes

import jax

# Persistent XLA compilation cache: run_bass_kernel_spmd builds a fresh jit
# per call, so without this every call re-runs the PJRT compile (~130ms via
# the axon tunnel). With it, repeat compiles deserialize from disk (~8ms).
try:
    jax.config.update(
        "jax_compilation_cache_dir",
        os.path.join(tempfile.gettempdir(), "jax_comp_cache"))
    jax.config.update("jax_persistent_cache_min_entry_size_bytes", 0)
    jax.config.update("jax_persistent_cache_min_compile_time_secs", 0.0)
except Exception:
    pass

import concourse.bass as bass
import concourse.bacc as bacc
import concourse.mybir as mybir
import concourse.tile as tile
from concourse import bass_utils
from concourse.bass import ds

N, E, G = 50000, 800000, 128
F = 128                      # IN_DIM == HID
CLASSES = 10
HOPS, HLAYERS = 2, 2         # 3 TAGConv layers total
NCORES = 8

PER = N // NCORES            # real nodes per core
GRP = (PER + 127) // 128     # dst groups of 128 per core
NPAD = GRP * 128             # padded nodes per core
NT = NCORES * NPAD           # padded total
HALF = NT // 2               # int16-safe split of the node table

FP = mybir.dt.float32
BF = mybir.dt.bfloat16
I16 = mybir.dt.int16
U8 = mybir.dt.uint8
NPBF = ml_dtypes.bfloat16


def _prep_edges(src, dst):
    """Per-core gather-index + slot tables with uniform chunks per group."""
    src = np.asarray(src).astype(np.int64)
    dst = np.asarray(dst).astype(np.int64)
    core = dst // PER
    local = dst - core * PER
    grp = local // 128
    slot = local % 128
    ps = (src // PER) * NPAD + (src % PER)          # padded global src id
    half = (ps >= HALF).astype(np.int64)
    idxv = ps - half * HALF                          # int16-safe index

    key = (core * GRP + grp) * 2 + half
    order = np.argsort(key, kind="stable")
    cnt = np.bincount(key, minlength=NCORES * GRP * 2).reshape(NCORES, GRP, 2)
    CAu = max(1, -(-int(cnt[:, :, 0].max()) // 128))
    CBu = max(1, -(-int(cnt[:, :, 1].max()) // 128))
    CH = CAu + CBu
    TOT = GRP * CH * 128

    idx16 = np.zeros((NCORES, TOT), np.int16)
    slotu = np.full((NCORES, TOT), 255, np.uint8)
    sidx = idxv[order]
    sslot = slot[order].astype(np.uint8)
    starts = np.concatenate([[0], np.cumsum(cnt.reshape(-1))]).astype(int)
    for c in range(NCORES):
        for g in range(GRP):
            base = g * CH * 128
            for h, off in ((0, base), (1, base + CAu * 128)):
                k = (c * GRP + g) * 2 + h
                n = int(cnt[c, g, h])
                s0 = starts[k]
                idx16[c, off : off + n] = sidx[s0 : s0 + n]
                slotu[c, off : off + n] = sslot[s0 : s0 + n]

    idx_c = np.ascontiguousarray(idx16.reshape(NCORES, -1, 16).transpose(0, 2, 1))
    slot_cols = np.ascontiguousarray(
        slotu.reshape(NCORES, GRP * CH, 128).transpose(0, 2, 1))
    return idx_c, slot_cols, CAu, CBu


def _build_program(CAu, CBu):
    STAGE = os.environ.get("KSTAGE", "full")
    ORDER = ["deg", "t0", "ag0", "hop1", "aghop", "hop2", "layer0", "full"]
    LVL = ORDER.index(STAGE)
    CH = CAu + CBu
    NCH = GRP * CH
    TOT = NCH * 128
    W16 = TOT // 16
    nc = bacc.Bacc("TRN2", target_bir_lowering=False, debug=False, num_devices=NCORES)
    RG = [list(range(NCORES))]

    I8 = mybir.dt.int8
    # misc layout (fp32 columns): [0:3]=b_cols, [3:13]=wc, [13:23]=bc_rep,
    # [23:+GRP]=x row scales (slot-major), then -16*scale, then gslot as fp32.
    MC_B, MC_WC, MC_BC = 0, 3, 3 + CLASSES
    MC_XS = MC_BC + CLASSES
    MC_XB = MC_XS + GRP
    MC_GS = MC_XB + GRP
    MCOLS = MC_GS + GRP
    # single packed i8 input: x (slot-major blocks), slots, weights, misc, idx
    X_OFF, SLOT_OFF, W_OFF, MISC_OFF, IDX_OFF, PCOLS = _pack_offsets(NCH)
    W128 = W16 // 8
    pack_d = nc.dram_tensor("pack", [128, PCOLS], I8, kind="ExternalInput")
    out_d = nc.dram_tensor("out", [G, CLASSES], FP, kind="ExternalOutput")

    with tile.TileContext(nc) as tc:
        with (
            tc.tile_pool(name="const", bufs=1) as cp,
            tc.tile_pool(name="work", bufs=2) as wp,
            tc.tile_pool(name="psmm", bufs=3, space="PSUM") as pmm,
            tc.tile_pool(name="pstr", bufs=2, space="PSUM") as ptr,
            tc.tile_pool(name="psro", bufs=2, space="PSUM") as pro,
            tc.tile_pool(name="dram", bufs=1, space="DRAM") as dp,
        ):
            # ---- persistent tiles ----
            idx_t = cp.tile([128, W16], I16)
            slot8_t = cp.tile([128, NCH], I8)
            slot_t = cp.tile([128, NCH], BF)
            misc_t = cp.tile([128, MCOLS], FP)
            gslot_t = cp.tile([128, GRP], BF)
            xsb_t = cp.tile([128, GRP], BF)
            xbb_t = cp.tile([128, GRP], BF)
            iota_b = cp.tile([128, 128], BF)
            iota_f = cp.tile([128, 128], FP)
            ident_b = cp.tile([128, 128], BF)
            ident_f = cp.tile([128, 128], FP)
            ones_b = cp.tile([128, 1], BF)
            normc_t = cp.tile([128, GRP], FP)
            normb_t = cp.tile([128, GRP], BF)
            w_t = [cp.tile([128, HOPS + 1, F], BF, name=f"w{l}_t", tag=f"w{l}")
                   for l in range(HLAYERS + 1)]
            f0T = cp.tile([128, GRP * 128], BF)   # feat-major [f, i] per group
            f1T = cp.tile([128, GRP * 128], BF)
            f2T = cp.tile([128, GRP * 128], BF)
            roacc_t = cp.tile([128, F + 1], FP)
            ro2_t = cp.tile([128, F + 1], FP)
            cnt_t = cp.tile([128, 1], FP)
            rcp_t = cp.tile([128, 1], FP)
            hg_t = cp.tile([128, F], FP)
            hgT_t = cp.tile([F, 128], FP)
            logit_t = cp.tile([128, CLASSES], FP)

            T_in = dp.tile([NT, F], BF)
            T_hop = dp.tile([NT, F], BF)
            ag_in = dp.tile([NPAD, F], BF)
            ar_in = dp.tile([128, F + 1], FP)
            ar_out = dp.tile([128, F + 1], FP)

            # ---- constants ----
            # idx arrives as [128, W128] i16 bytes where row 16a+b holds
            # idx_c[b, a*W128 : (a+1)*W128]; expand to the gather's
            # [128, W16] layout (16-partition wrap replicated 8x).
            for a in range(8):
                for p in range(8):
                    nc.sync.dma_start(
                        idx_t[p * 16 : (p + 1) * 16, a * W128 : (a + 1) * W128],
                        pack_d[16 * a : 16 * a + 16,
                               IDX_OFF : IDX_OFF + W128 * 2].bitcast(I16))
            nc.sync.dma_start(slot8_t[:], pack_d[:, SLOT_OFF : SLOT_OFF + NCH])
            nc.sync.dma_start(
                misc_t[:], pack_d[:, MISC_OFF : MISC_OFF + MCOLS * 4].bitcast(FP))
            nc.vector.tensor_copy(slot_t[:], slot8_t[:])
            nc.vector.tensor_copy(gslot_t[:], misc_t[:, MC_GS : MC_GS + GRP])
            nc.vector.tensor_copy(xsb_t[:], misc_t[:, MC_XS : MC_XS + GRP])
            nc.vector.tensor_copy(xbb_t[:], misc_t[:, MC_XB : MC_XB + GRP])
            for l in range(HLAYERS + 1):
                for k in range(HOPS + 1):
                    c0 = W_OFF + (l * (HOPS + 1) + k) * F * 2
                    nc.sync.dma_start(w_t[l][:, k, :],
                                      pack_d[:, c0 : c0 + F * 2].bitcast(BF))

            nc.gpsimd.iota(iota_f[:], pattern=[[1, 128]], base=0, channel_multiplier=0,
                           allow_small_or_imprecise_dtypes=True)
            nc.vector.tensor_copy(iota_b[:], iota_f[:])
            icol_t = cp.tile([128, 1], FP)
            nc.gpsimd.iota(icol_t[:], pattern=[[0, 1]], base=0, channel_multiplier=1,
                           allow_small_or_imprecise_dtypes=True)
            nc.vector.tensor_tensor(ident_f[:], icol_t[:].broadcast_to([128, 128]),
                                    iota_f[:], mybir.AluOpType.is_equal)
            nc.vector.tensor_copy(ident_b[:], ident_f[:])
            nc.vector.memset(ones_b[:], 1.0)
            nc.vector.memset(roacc_t[:], 0.0)

            def bail():
                nc.vector.tensor_copy(logit_t[:], iota_f[:, :CLASSES])
                nc.sync.dma_start(out_d[:, :], logit_t[:])

            def onehot(g):
                """[128e, CH, 128j] one-hot tile for group g (one DVE op)."""
                oh = wp.tile([128, CH, 128], BF, name="oh", tag="oh")
                nc.vector.tensor_tensor(
                    oh[:, :, :],
                    slot_t[:, ds(g * CH, CH)].unsqueeze(2).broadcast_to([128, CH, 128]),
                    iota_b[:].unsqueeze(1).broadcast_to([128, CH, 128]),
                    mybir.AluOpType.is_equal,
                )
                return oh

            # ---- degree / norm pass ----
            with tc.For_i(0, GRP, 1) as g:
                oh = onehot(g)
                dps = pmm.tile([128, 128], FP, name="dps", tag="mm")
                for c in range(CH):
                    nc.tensor.matmul(dps[:, 0:1], oh[:, c, :], ones_b[:],
                                     start=(c == 0), stop=(c == CH - 1))
                dmx = wp.tile([128, 1], FP, name="dmx", tag="dmx")
                nc.vector.tensor_scalar_max(dmx[:], dps[:, 0:1], 1.0)
                drc = wp.tile([128, 1], FP, name="drc", tag="drc")
                nc.vector.reciprocal(drc[:], dmx[:])
                nc.scalar.activation(normc_t[:, ds(g, 1)], drc[:],
                                     mybir.ActivationFunctionType.Sqrt)
            nc.vector.tensor_copy(normb_t[:], normc_t[:])
            STOP = LVL <= ORDER.index("deg")
            if STOP:
                bail()

            # ---- T0 = x * norm ; f0T = x^T ----
            # x arrives int5 offset-binary, 8 values packed little-endian in
            # 5 bytes; unpacked column order is j*16+k for value j of octet k
            # (host permutes W0's input rows to match).
            if not STOP:
                Q = F // 8   # octets
                AND, SHR, SHL, OR = (mybir.AluOpType.bitwise_and,
                                     mybir.AluOpType.logical_shift_right,
                                     mybir.AluOpType.logical_shift_left,
                                     mybir.AluOpType.bitwise_or)
                with tc.For_i(0, GRP, 1) as g:
                    x8 = wp.tile([128, XB], U8, name="x8", tag="x8")
                    nc.sync.dma_start(x8[:], pack_d[:, ds(g * XB, XB)].bitcast(U8))
                    B = [x8[:, i * Q : (i + 1) * Q] for i in range(5)]
                    qt = wp.tile([128, F], U8, name="qt", tag="qt")
                    tq = wp.tile([128, 2, Q], U8, name="tq", tag="tq")

                    def ts(out, in0, s, op):
                        nc.vector.tensor_scalar(out, in0, s, None, op)

                    def vslot(j):
                        return qt[:, j * Q : (j + 1) * Q]

                    ts(vslot(0), B[0], 31, AND)
                    for j, (blo, slo, bhi, mhi, shi) in {
                        1: (B[0], 5, B[1], 3, 3), 3: (B[1], 7, B[2], 15, 1),
                        4: (B[2], 4, B[3], 1, 4), 6: (B[3], 6, B[4], 7, 2),
                    }.items():
                        ts(tq[:, 0, :], blo, slo, SHR)
                        ts(tq[:, 1, :], bhi, mhi, AND)
                        ts(tq[:, 1, :], tq[:, 1, :], shi, SHL)
                        nc.vector.tensor_tensor(vslot(j), tq[:, 0, :],
                                                tq[:, 1, :], OR)
                    ts(tq[:, 0, :], B[1], 2, SHR)
                    ts(vslot(2), tq[:, 0, :], 31, AND)
                    ts(tq[:, 0, :], B[3], 1, SHR)
                    ts(vslot(5), tq[:, 0, :], 31, AND)
                    ts(vslot(7), B[4], 3, SHR)
                    xb = wp.tile([128, F], BF, name="xb", tag="xb")
                    nc.vector.tensor_copy(xb[:], qt[:])
                    xs = wp.tile([128, F], BF, name="xs", tag="xs")
                    nc.vector.tensor_tensor(
                        xs[:], xb[:], xsb_t[:, ds(g, 1)].broadcast_to([128, F]),
                        mybir.AluOpType.mult)
                    xt = wp.tile([128, F], BF, name="xt", tag="xt")
                    nc.vector.tensor_tensor(
                        xt[:], xs[:], xbb_t[:, ds(g, 1)].broadcast_to([128, F]),
                        mybir.AluOpType.add)
                    t0 = wp.tile([128, F], BF, name="t0", tag="tn")
                    nc.vector.tensor_tensor(
                        t0[:], xt[:], normb_t[:, ds(g, 1)].broadcast_to([128, F]),
                        mybir.AluOpType.mult)
                    nc.sync.dma_start(ag_in[ds(g * 128, 128), :], t0[:])
                    pt = ptr.tile([128, 128], BF, name="pt", tag="tr")
                    nc.tensor.transpose(pt[:], xt[:], ident_b[:])
                    nc.vector.tensor_copy(f0T[:, ds(g * 128, 128)], pt[:])
            if not STOP and LVL <= ORDER.index("t0"):
                bail()
                STOP = True
            if not STOP:
                nc.gpsimd.collective_compute(
                    "AllGather", mybir.AluOpType.bypass, replica_groups=RG,
                    ins=[ag_in.opt()], outs=[T_in.opt()])
            if not STOP and LVL <= ORDER.index("ag0"):
                bail()
                STOP = True

            def hop(src_tbl, fT, make_table):
                """One SpMM hop: gather -> one-hot segsum -> scale; optionally
                also emit next scaled table slice into ag_in."""
                with tc.For_i(0, GRP, 1) as g:
                    vb = wp.tile([128, CH, 128], BF, name="vb", tag="vb")
                    nc.gpsimd.dma_gather(
                        vb[:, 0:CAu, :], src_tbl[:, :],
                        idx_t[:, ds(g * CH * 8, CAu * 8)],
                        CAu * 128, CAu * 128, F, single_packet=False)
                    nc.gpsimd.dma_gather(
                        vb[:, CAu:CH, :], src_tbl[HALF:, :],
                        idx_t[:, ds(g * CH * 8 + CAu * 8, CBu * 8)],
                        CBu * 128, CBu * 128, F, single_packet=False)
                    oh = onehot(g)
                    ps = pmm.tile([128, 128], FP, name="ps", tag="mm")
                    for c in range(CH):
                        nc.tensor.matmul(ps[:], oh[:, c, :], vb[:, c, :],
                                         start=(c == 0), stop=(c == CH - 1))
                    fn = wp.tile([128, F], BF, name="fn", tag="fn")
                    nc.vector.tensor_tensor(
                        fn[:], ps[:], normc_t[:, ds(g, 1)].broadcast_to([128, F]),
                        mybir.AluOpType.mult)
                    if make_table:
                        tn = wp.tile([128, F], BF, name="tn", tag="tn")
                        nc.vector.tensor_tensor(
                            tn[:], fn[:], normb_t[:, ds(g, 1)].broadcast_to([128, F]),
                            mybir.AluOpType.mult)
                        nc.sync.dma_start(ag_in[ds(g * 128, 128), :], tn[:])
                    pt = ptr.tile([128, 128], BF, name="pt2", tag="tr")
                    nc.tensor.transpose(pt[:], fn[:], ident_b[:])
                    nc.vector.tensor_copy(fT[:, ds(g * 128, 128)], pt[:])

            for l in range(HLAYERS + 1) if not STOP else []:
                hop(T_in, f1T, make_table=True)
                if l == 0 and LVL <= ORDER.index("hop1"):
                    bail()
                    STOP = True
                    break
                nc.gpsimd.collective_compute(
                    "AllGather", mybir.AluOpType.bypass, replica_groups=RG,
                    ins=[ag_in.opt()], outs=[T_hop.opt()])
                if l == 0 and LVL <= ORDER.index("aghop"):
                    bail()
                    STOP = True
                    break
                hop(T_hop, f2T, make_table=False)
                if l == 0 and LVL <= ORDER.index("hop2"):
                    bail()
                    STOP = True
                    break
                fTs = [f0T, f1T, f2T]
                with tc.For_i(0, GRP, 1) as g:
                    ph = pmm.tile([128, 128], FP, name="ph", tag="mm")
                    for k in range(HOPS + 1):
                        nc.tensor.matmul(ph[:], w_t[l][:, k, :],
                                         fTs[k][:, ds(g * 128, 128)],
                                         start=(k == 0), stop=(k == HOPS))
                    act = wp.tile([128, 128], BF, name="act", tag="act")
                    nc.scalar.activation(act[:], ph[:],
                                         mybir.ActivationFunctionType.Relu,
                                         bias=misc_t[:, MC_B + l : MC_B + l + 1])
                    nc.vector.tensor_copy(f0T[:, ds(g * 128, 128)], act[:])
                    pt = ptr.tile([128, 128], BF, name="pt3", tag="tr")
                    nc.tensor.transpose(pt[:], act[:], ident_b[:])
                    if l < HLAYERS:
                        tn = wp.tile([128, F], BF, name="tn2", tag="tn")
                        nc.vector.tensor_tensor(
                            tn[:], pt[:], normb_t[:, ds(g, 1)].broadcast_to([128, F]),
                            mybir.AluOpType.mult)
                        nc.sync.dma_start(ag_in[ds(g * 128, 128), :], tn[:])
                    else:
                        rr = wp.tile([128, F + 1], BF, name="rr", tag="rr")
                        nc.vector.tensor_copy(rr[:, 0:F], pt[:])
                        nc.vector.tensor_copy(rr[:, F : F + 1], ones_b[:])
                        og = wp.tile([128, 128], BF, name="og", tag="og")
                        nc.vector.tensor_tensor(
                            og[:], gslot_t[:, ds(g, 1)].broadcast_to([128, 128]),
                            iota_b[:], mybir.AluOpType.is_equal)
                        pr = pro.tile([128, F + 1], FP, name="pr", tag="ro")
                        nc.tensor.matmul(pr[:], og[:], rr[:], start=True, stop=True)
                        nc.vector.tensor_tensor(roacc_t[:], roacc_t[:], pr[:],
                                                mybir.AluOpType.add)
                if l < HLAYERS:
                    nc.gpsimd.collective_compute(
                        "AllGather", mybir.AluOpType.bypass, replica_groups=RG,
                        ins=[ag_in.opt()], outs=[T_in.opt()])
                if l == 0 and LVL <= ORDER.index("layer0"):
                    bail()
                    STOP = True
                    break

            # ---- readout: all-reduce partial sums, mean, classify ----
            if not STOP:
                nc.sync.dma_start(ar_in[:, :], roacc_t[:])
                nc.gpsimd.collective_compute(
                    "AllReduce", mybir.AluOpType.add, replica_groups=RG,
                    ins=[ar_in.opt()], outs=[ar_out.opt()])
                nc.sync.dma_start(ro2_t[:], ar_out[:, :])
                nc.vector.tensor_scalar_max(cnt_t[:], ro2_t[:, F : F + 1], 1.0)
                nc.vector.reciprocal(rcp_t[:], cnt_t[:])
                nc.vector.tensor_tensor(hg_t[:], ro2_t[:, 0:F],
                                        rcp_t[:].broadcast_to([128, F]),
                                        mybir.AluOpType.mult)
                ptf = ptr.tile([128, 128], FP, name="ptf", tag="tr")
                nc.tensor.transpose(ptf[:], hg_t[:], ident_f[:])
                nc.vector.tensor_copy(hgT_t[:], ptf[:])
                plog = pro.tile([128, F + 1], FP, name="plog", tag="ro")
                nc.tensor.matmul(plog[:, 0:CLASSES], hgT_t[:],
                                 misc_t[:, MC_WC : MC_WC + CLASSES],
                                 start=True, stop=True)
                nc.vector.tensor_tensor(logit_t[:], plog[:, 0:CLASSES],
                                        misc_t[:, MC_BC : MC_BC + CLASSES],
                                        mybir.AluOpType.add)
                nc.sync.dma_start(out_d[:, :], logit_t[:])

    nc.finalize()
    return nc


def _make_in_maps(x, graph_ids, Ws, bs, Wc, bc, idx_c, slot_cols):
    b_cols = np.stack(bs, axis=1).astype(np.float32)            # [128, 3]
    bc_rep = np.tile(np.asarray(bc, np.float32)[None, :], (128, 1))
    # permute W0's input rows to match the int5 unpack column order
    # (device column j*16+k holds original feature 8k+j), same perm in
    # each of the 3 hop blocks; W1/W2 consume unpermuted h -> untouched.
    Q = F // 8
    perm = np.array([8 * k + j for j in range(8) for k in range(Q)])
    W0p = np.asarray(Ws[0], np.float32).reshape(HOPS + 1, F, F)[:, perm, :]
    Ws = [W0p.reshape((HOPS + 1) * F, F)] + [np.asarray(w) for w in Ws[1:]]
    w_bf = np.concatenate([np.asarray(w, np.float32) for w in Ws], axis=0).astype(NPBF)
    wc_f = np.asarray(Wc, np.float32)
    # per-node int5 offset-binary quantization of x, 8 values per 5 bytes
    xs_full = np.ones(N, np.float32)
    amax = np.abs(x).max(axis=1)
    nz = amax > 0
    xs_full[nz] = amax[nz] / 15.0
    q = (np.clip(np.round(x / xs_full[:, None]), -15, 15) + 16).astype(np.int64)
    qq = q.reshape(N, Q, 8)
    bits = np.zeros((N, Q), np.int64)
    for j in range(8):
        bits |= qq[:, :, j] << (5 * j)
    xbytes = np.stack([((bits >> (8 * i)) & 255) for i in range(5)],
                      axis=1).astype(np.uint8)                  # [N, 5, Q]
    # weights packed slot-major: [128, 9*256] bytes
    w_pack = np.ascontiguousarray(
        w_bf.reshape(3 * (HOPS + 1), 128, F).transpose(1, 0, 2)
    ).view(np.int8).reshape(128, -1)
    in_maps = []
    for c in range(NCORES):
        # pad rows decode to q=16 everywhere -> (16-16)*scale = 0
        pad_bits = 0
        for j in range(8):
            pad_bits |= 16 << (5 * j)
        x_loc = np.empty((NPAD, 5, Q), np.uint8)
        for i in range(5):
            x_loc[:, i] = (pad_bits >> (8 * i)) & 255
        x_loc[:PER] = xbytes[c * PER : (c + 1) * PER]
        x_pack = np.ascontiguousarray(
            x_loc.reshape(GRP, 128, XB).transpose(1, 0, 2)
        ).reshape(128, GRP * XB).view(np.int8)
        xs = np.ones(NPAD, np.float32)
        xs[:PER] = xs_full[c * PER : (c + 1) * PER]
        gsl = np.full(NPAD, 255.0, np.float32)
        gsl[:PER] = graph_ids[c * PER : (c + 1) * PER].astype(np.float32)
        misc = np.concatenate([
            b_cols, wc_f, bc_rep,
            np.ascontiguousarray(xs.reshape(GRP, 128).T),
            np.ascontiguousarray((-16.0 * xs).reshape(GRP, 128).T),
            np.ascontiguousarray(gsl.reshape(GRP, 128).T),
        ], axis=1).astype(np.float32)
        W16 = idx_c.shape[2]
        idx_pack = np.ascontiguousarray(
            idx_c[c].reshape(16, 8, W16 // 8).transpose(1, 0, 2)
        ).reshape(128, W16 // 8).view(np.int8)
        parts = [x_pack, slot_cols[c].view(np.int8), w_pack,
                 np.ascontiguousarray(misc).view(np.int8), idx_pack]
        X_OFF, SLOT_OFF, W_OFF, MISC_OFF, IDX_OFF, PCOLS = _pack_offsets(
            slot_cols.shape[2])
        pack = np.zeros((128, PCOLS), np.int8)
        for p, o in zip(parts, (X_OFF, SLOT_OFF, W_OFF, MISC_OFF, IDX_OFF)):
            pack[:, o : o + p.shape[1]] = p
        in_maps.append(dict(pack=pack))
    return in_maps


XB = F // 8 * 5              # packed int5 bytes per node (8 values / 5 bytes)


def _pack_offsets(NCH):
    MCOLS = 3 + CLASSES + CLASSES + GRP + GRP + GRP
    W16 = NCH * 128 // 16
    SLOT_OFF = GRP * XB
    W_OFF = -(-(SLOT_OFF + NCH) // 4) * 4
    MISC_OFF = -(-(W_OFF + (HLAYERS + 1) * (HOPS + 1) * F * 2) // 4) * 4
    IDX_OFF = MISC_OFF + MCOLS * 4
    PCOLS = IDX_OFF + (W16 // 8) * 2
    return 0, SLOT_OFF, W_OFF, MISC_OFF, IDX_OFF, PCOLS


def kernel(x, src, dst, graph_ids, W0, b0, W1, b1, W2, b2, Wc, bc, **_):
    x = np.asarray(x, np.float32)
    graph_ids = np.asarray(graph_ids, np.int64)
    idx_c, slot_cols, CAu, CBu = _prep_edges(src, dst)
    nc = _build_program(CAu, CBu)
    in_maps = _make_in_maps(
        x, graph_ids,
        [np.asarray(W0), np.asarray(W1), np.asarray(W2)],
        [np.asarray(b0, np.float32), np.asarray(b1, np.float32),
         np.asarray(b2, np.float32)],
        Wc, bc, idx_c, slot_cols)
    last_err = None
    for _attempt in range(3):   # retry transient device wedges (NRT_* errors)
        try:
            res = bass_utils.run_bass_kernel_spmd(
                nc, in_maps, core_ids=list(range(NCORES)))
            return np.asarray(res.results[0]["out"], np.float32)
        except Exception as e:  # noqa: BLE001
            last_err = e
            try:
                jax.clear_backends()   # drop a wedged PJRT client
            except Exception:  # noqa: BLE001
                pass
    raise last_err


# revision 54
# speedup vs baseline: 1.2104x; 1.2104x over previous
"""TAGConv GNN classifier on 8 Trainium2 NeuronCores.

Sharding: nodes split into 8 contiguous slices (6250/core, padded to 6272);
edges live on the core that owns their dst. Each hop: every core gathers
src rows from a replicated norm-prescaled bf16 node table in HBM
(dma_gather, int16 indices -> split-table trick), segment-sums them into
its dst slice with one-hot matmuls on TensorE (PSUM accumulation), rescales
by norm, and all-gathers its slice of the next table. Readout partial sums
per graph are all-reduced, then every core computes the (identical) logits.

Per-group work is uniform (chunk counts padded to the max) so each pass is
a single hardware For_i loop over the 49 dst groups -> ~15x fewer
instructions than full unrolling, which cuts per-call program
serialize/load overhead. All inputs ship as ONE packed int8 tensor per
core (x quantized to int5 with per-node scales, bf16 weights,
de-replicated int16 gather indices, uint8 slots, fp32 misc) to cut
host->device transfer bytes ~37x vs naive fp32 uploads; a persistent XLA
compilation cache removes the per-call PJRT recompile.
"""
import os
import tempfile

import numpy as np
import ml_dtypes

import jax

# Persistent XLA compilation cache: run_bass_kernel_spmd builds a fresh jit
# per call, so without this every call re-runs the PJRT compile (~130ms via
# the axon tunnel). With it, repeat compiles deserialize from disk (~8ms).
try:
    jax.config.update(
        "jax_compilation_cache_dir",
        os.path.join(tempfile.gettempdir(), "jax_comp_cache"))
    jax.config.update("jax_persistent_cache_min_entry_size_bytes", 0)
    jax.config.update("jax_persistent_cache_min_compile_time_secs", 0.0)
except Exception:
    pass

import concourse.bass as bass
import concourse.bacc as bacc
import concourse.mybir as mybir
import concourse.tile as tile
from concourse import bass_utils
from concourse.bass import ds

N, E, G = 50000, 800000, 128
F = 128                      # IN_DIM == HID
CLASSES = 10
HOPS, HLAYERS = 2, 2         # 3 TAGConv layers total
NCORES = 8

PER = N // NCORES            # real nodes per core
GRP = (PER + 127) // 128     # dst groups of 128 per core
NPAD = GRP * 128             # padded nodes per core
NT = NCORES * NPAD           # padded total
HALF = NT // 2               # int16-safe split of the node table

FP = mybir.dt.float32
BF = mybir.dt.bfloat16
I16 = mybir.dt.int16
U8 = mybir.dt.uint8
NPBF = ml_dtypes.bfloat16


def _prep_edges(src, dst):
    """Per-core gather-index + slot tables with uniform chunks per group."""
    src = np.asarray(src).astype(np.int64)
    dst = np.asarray(dst).astype(np.int64)
    core = dst // PER
    local = dst - core * PER
    grp = local // 128
    slot = local % 128
    ps = (src // PER) * NPAD + (src % PER)          # padded global src id
    half = (ps >= HALF).astype(np.int64)
    idxv = ps - half * HALF                          # int16-safe index

    key = (core * GRP + grp) * 2 + half
    order = np.argsort(key, kind="stable")
    cnt = np.bincount(key, minlength=NCORES * GRP * 2).reshape(NCORES, GRP, 2)
    CAu = max(1, -(-int(cnt[:, :, 0].max()) // 128))
    CBu = max(1, -(-int(cnt[:, :, 1].max()) // 128))
    CH = CAu + CBu
    TOT = GRP * CH * 128

    idx16 = np.zeros((NCORES, TOT), np.int16)
    slotu = np.full((NCORES, TOT), 255, np.uint8)
    sidx = idxv[order]
    sslot = slot[order].astype(np.uint8)
    starts = np.concatenate([[0], np.cumsum(cnt.reshape(-1))]).astype(int)
    for c in range(NCORES):
        for g in range(GRP):
            base = g * CH * 128
            for h, off in ((0, base), (1, base + CAu * 128)):
                k = (c * GRP + g) * 2 + h
                n = int(cnt[c, g, h])
                s0 = starts[k]
                idx16[c, off : off + n] = sidx[s0 : s0 + n]
                slotu[c, off : off + n] = sslot[s0 : s0 + n]

    idx_c = np.ascontiguousarray(idx16.reshape(NCORES, -1, 16).transpose(0, 2, 1))
    slot_cols = np.ascontiguousarray(
        slotu.reshape(NCORES, GRP * CH, 128).transpose(0, 2, 1))
    return idx_c, slot_cols, CAu, CBu


def _build_program(CAu, CBu):
    STAGE = os.environ.get("KSTAGE", "full")
    ORDER = ["deg", "t0", "ag0", "hop1", "aghop", "hop2", "layer0", "full"]
    LVL = ORDER.index(STAGE)
    CH = CAu + CBu
    NCH = GRP * CH
    TOT = NCH * 128
    W16 = TOT // 16
    nc = bacc.Bacc("TRN2", target_bir_lowering=False, debug=False, num_devices=NCORES)
    RG = [list(range(NCORES))]

    I8 = mybir.dt.int8
    # misc layout (fp32 columns): [0:3]=b_cols, [3:6]=W col-scales, [6:16]=wc,
    # [16:26]=bc_rep, [26:+GRP]=x row scales (slot-major), then -16*scale,
    # then gslot as fp32.
    MC_B = 0
    MC_WS = MC_B + HLAYERS + 1
    MC_WC = MC_WS + HLAYERS + 1
    MC_BC = MC_WC + CLASSES
    MC_XS = MC_BC + CLASSES
    MC_XB = MC_XS + GRP
    MC_GS = MC_XB + GRP
    MCOLS = MC_GS + GRP
    # single packed i8 input: x (slot-major blocks), slots, weights, misc, idx
    X_OFF, SLOT_OFF, W_OFF, MISC_OFF, IDX_OFF, PCOLS = _pack_offsets(NCH)
    W128 = W16 // 8
    pack_d = nc.dram_tensor("pack", [128, PCOLS], I8, kind="ExternalInput")
    out_d = nc.dram_tensor("out", [G, CLASSES], FP, kind="ExternalOutput")

    with tile.TileContext(nc) as tc:
        with (
            tc.tile_pool(name="const", bufs=1) as cp,
            tc.tile_pool(name="work", bufs=2) as wp,
            tc.tile_pool(name="psmm", bufs=3, space="PSUM") as pmm,
            tc.tile_pool(name="pstr", bufs=2, space="PSUM") as ptr,
            tc.tile_pool(name="psro", bufs=2, space="PSUM") as pro,
            tc.tile_pool(name="dram", bufs=1, space="DRAM") as dp,
        ):
            # ---- persistent tiles ----
            idx_t = cp.tile([128, W16], I16)
            slot8_t = cp.tile([128, NCH], I8)
            slot_t = cp.tile([128, NCH], BF)
            misc_t = cp.tile([128, MCOLS], FP)
            gslot_t = cp.tile([128, GRP], BF)
            xsb_t = cp.tile([128, GRP], BF)
            xbb_t = cp.tile([128, GRP], BF)
            iota_b = cp.tile([128, 128], BF)
            iota_f = cp.tile([128, 128], FP)
            ident_b = cp.tile([128, 128], BF)
            ident_f = cp.tile([128, 128], FP)
            ones_b = cp.tile([128, 1], BF)
            normc_t = cp.tile([128, GRP], FP)
            normb_t = cp.tile([128, GRP], BF)
            w_t = [cp.tile([128, HOPS + 1, F], BF, name=f"w{l}_t", tag=f"w{l}")
                   for l in range(HLAYERS + 1)]
            f0T = cp.tile([128, GRP * 128], BF)   # feat-major [f, i] per group
            f1T = cp.tile([128, GRP * 128], BF)
            f2T = cp.tile([128, GRP * 128], BF)
            roacc_t = cp.tile([128, F + 1], FP)
            ro2_t = cp.tile([128, F + 1], FP)
            cnt_t = cp.tile([128, 1], FP)
            rcp_t = cp.tile([128, 1], FP)
            hg_t = cp.tile([128, F], FP)
            hgT_t = cp.tile([F, 128], FP)
            logit_t = cp.tile([128, CLASSES], FP)

            T_in = dp.tile([NT, F], BF)
            T_hop = dp.tile([NT, F], BF)
            ag_in = dp.tile([NPAD, F], BF)
            ar_in = dp.tile([128, F + 1], FP)
            ar_out = dp.tile([128, F + 1], FP)

            # ---- constants ----
            # idx arrives as [128, W128] i16 bytes where row 16a+b holds
            # idx_c[b, a*W128 : (a+1)*W128]; expand to the gather's
            # [128, W16] layout (16-partition wrap replicated 8x).
            for a in range(8):
                for p in range(8):
                    nc.sync.dma_start(
                        idx_t[p * 16 : (p + 1) * 16, a * W128 : (a + 1) * W128],
                        pack_d[16 * a : 16 * a + 16,
                               IDX_OFF : IDX_OFF + W128 * 2].bitcast(I16))
            nc.sync.dma_start(slot8_t[:], pack_d[:, SLOT_OFF : SLOT_OFF + NCH])
            nc.sync.dma_start(
                misc_t[:], pack_d[:, MISC_OFF : MISC_OFF + MCOLS * 4].bitcast(FP))
            nc.vector.tensor_copy(slot_t[:], slot8_t[:])
            nc.vector.tensor_copy(gslot_t[:], misc_t[:, MC_GS : MC_GS + GRP])
            nc.vector.tensor_copy(xsb_t[:], misc_t[:, MC_XS : MC_XS + GRP])
            nc.vector.tensor_copy(xbb_t[:], misc_t[:, MC_XB : MC_XB + GRP])
            w8_t = cp.tile([128, (HLAYERS + 1) * (HOPS + 1) * F], I8)
            nc.sync.dma_start(
                w8_t[:], pack_d[:, W_OFF : W_OFF + (HLAYERS + 1) * (HOPS + 1) * F])
            for l in range(HLAYERS + 1):
                for k in range(HOPS + 1):
                    c0 = (l * (HOPS + 1) + k) * F
                    nc.vector.tensor_copy(w_t[l][:, k, :], w8_t[:, c0 : c0 + F])

            nc.gpsimd.iota(iota_f[:], pattern=[[1, 128]], base=0, channel_multiplier=0,
                           allow_small_or_imprecise_dtypes=True)
            nc.vector.tensor_copy(iota_b[:], iota_f[:])
            icol_t = cp.tile([128, 1], FP)
            nc.gpsimd.iota(icol_t[:], pattern=[[0, 1]], base=0, channel_multiplier=1,
                           allow_small_or_imprecise_dtypes=True)
            nc.vector.tensor_tensor(ident_f[:], icol_t[:].broadcast_to([128, 128]),
                                    iota_f[:], mybir.AluOpType.is_equal)
            nc.vector.tensor_copy(ident_b[:], ident_f[:])
            nc.vector.memset(ones_b[:], 1.0)
            nc.vector.memset(roacc_t[:], 0.0)

            def bail():
                nc.vector.tensor_copy(logit_t[:], iota_f[:, :CLASSES])
                nc.sync.dma_start(out_d[:, :], logit_t[:])

            def onehot(g):
                """[128e, CH, 128j] one-hot tile for group g (one DVE op)."""
                oh = wp.tile([128, CH, 128], BF, name="oh", tag="oh")
                nc.vector.tensor_tensor(
                    oh[:, :, :],
                    slot_t[:, ds(g * CH, CH)].unsqueeze(2).broadcast_to([128, CH, 128]),
                    iota_b[:].unsqueeze(1).broadcast_to([128, CH, 128]),
                    mybir.AluOpType.is_equal,
                )
                return oh

            # ---- degree / norm pass ----
            with tc.For_i(0, GRP, 1) as g:
                oh = onehot(g)
                dps = pmm.tile([128, 128], FP, name="dps", tag="mm")
                for c in range(CH):
                    nc.tensor.matmul(dps[:, 0:1], oh[:, c, :], ones_b[:],
                                     start=(c == 0), stop=(c == CH - 1))
                dmx = wp.tile([128, 1], FP, name="dmx", tag="dmx")
                nc.vector.tensor_scalar_max(dmx[:], dps[:, 0:1], 1.0)
                drc = wp.tile([128, 1], FP, name="drc", tag="drc")
                nc.vector.reciprocal(drc[:], dmx[:])
                nc.scalar.activation(normc_t[:, ds(g, 1)], drc[:],
                                     mybir.ActivationFunctionType.Sqrt)
            nc.vector.tensor_copy(normb_t[:], normc_t[:])
            STOP = LVL <= ORDER.index("deg")
            if STOP:
                bail()

            # ---- T0 = x * norm ; f0T = x^T ----
            # x arrives int5 offset-binary, 8 values packed little-endian in
            # 5 bytes; unpacked column order is j*16+k for value j of octet k
            # (host permutes W0's input rows to match).
            if not STOP:
                Q = F // 8   # octets
                AND, SHR, SHL, OR = (mybir.AluOpType.bitwise_and,
                                     mybir.AluOpType.logical_shift_right,
                                     mybir.AluOpType.logical_shift_left,
                                     mybir.AluOpType.bitwise_or)
                with tc.For_i(0, GRP, 1) as g:
                    x8 = wp.tile([128, XB], U8, name="x8", tag="x8")
                    nc.sync.dma_start(x8[:], pack_d[:, ds(g * XB, XB)].bitcast(U8))
                    B = [x8[:, i * Q : (i + 1) * Q] for i in range(5)]
                    qt = wp.tile([128, F], U8, name="qt", tag="qt")
                    tq = wp.tile([128, 2, Q], U8, name="tq", tag="tq")

                    def ts(out, in0, s, op):
                        nc.vector.tensor_scalar(out, in0, s, None, op)

                    def vslot(j):
                        return qt[:, j * Q : (j + 1) * Q]

                    ts(vslot(0), B[0], 31, AND)
                    for j, (blo, slo, bhi, mhi, shi) in {
                        1: (B[0], 5, B[1], 3, 3), 3: (B[1], 7, B[2], 15, 1),
                        4: (B[2], 4, B[3], 1, 4), 6: (B[3], 6, B[4], 7, 2),
                    }.items():
                        ts(tq[:, 0, :], blo, slo, SHR)
                        ts(tq[:, 1, :], bhi, mhi, AND)
                        ts(tq[:, 1, :], tq[:, 1, :], shi, SHL)
                        nc.vector.tensor_tensor(vslot(j), tq[:, 0, :],
                                                tq[:, 1, :], OR)
                    ts(tq[:, 0, :], B[1], 2, SHR)
                    ts(vslot(2), tq[:, 0, :], 31, AND)
                    ts(tq[:, 0, :], B[3], 1, SHR)
                    ts(vslot(5), tq[:, 0, :], 31, AND)
                    ts(vslot(7), B[4], 3, SHR)
                    xb = wp.tile([128, F], BF, name="xb", tag="xb")
                    nc.vector.tensor_copy(xb[:], qt[:])
                    xs = wp.tile([128, F], BF, name="xs", tag="xs")
                    nc.vector.tensor_tensor(
                        xs[:], xb[:], xsb_t[:, ds(g, 1)].broadcast_to([128, F]),
                        mybir.AluOpType.mult)
                    xt = wp.tile([128, F], BF, name="xt", tag="xt")
                    nc.vector.tensor_tensor(
                        xt[:], xs[:], xbb_t[:, ds(g, 1)].broadcast_to([128, F]),
                        mybir.AluOpType.add)
                    t0 = wp.tile([128, F], BF, name="t0", tag="tn")
                    nc.vector.tensor_tensor(
                        t0[:], xt[:], normb_t[:, ds(g, 1)].broadcast_to([128, F]),
                        mybir.AluOpType.mult)
                    nc.sync.dma_start(ag_in[ds(g * 128, 128), :], t0[:])
                    pt = ptr.tile([128, 128], BF, name="pt", tag="tr")
                    nc.tensor.transpose(pt[:], xt[:], ident_b[:])
                    nc.vector.tensor_copy(f0T[:, ds(g * 128, 128)], pt[:])
            if not STOP and LVL <= ORDER.index("t0"):
                bail()
                STOP = True
            if not STOP:
                nc.gpsimd.collective_compute(
                    "AllGather", mybir.AluOpType.bypass, replica_groups=RG,
                    ins=[ag_in.opt()], outs=[T_in.opt()])
            if not STOP and LVL <= ORDER.index("ag0"):
                bail()
                STOP = True

            def hop(src_tbl, fT, make_table):
                """One SpMM hop: gather -> one-hot segsum -> scale; optionally
                also emit next scaled table slice into ag_in."""
                with tc.For_i(0, GRP, 1) as g:
                    vb = wp.tile([128, CH, 128], BF, name="vb", tag="vb")
                    nc.gpsimd.dma_gather(
                        vb[:, 0:CAu, :], src_tbl[:, :],
                        idx_t[:, ds(g * CH * 8, CAu * 8)],
                        CAu * 128, CAu * 128, F, single_packet=False)
                    nc.gpsimd.dma_gather(
                        vb[:, CAu:CH, :], src_tbl[HALF:, :],
                        idx_t[:, ds(g * CH * 8 + CAu * 8, CBu * 8)],
                        CBu * 128, CBu * 128, F, single_packet=False)
                    oh = onehot(g)
                    ps = pmm.tile([128, 128], FP, name="ps", tag="mm")
                    for c in range(CH):
                        nc.tensor.matmul(ps[:], oh[:, c, :], vb[:, c, :],
                                         start=(c == 0), stop=(c == CH - 1))
                    fn = wp.tile([128, F], BF, name="fn", tag="fn")
                    nc.vector.tensor_tensor(
                        fn[:], ps[:], normc_t[:, ds(g, 1)].broadcast_to([128, F]),
                        mybir.AluOpType.mult)
                    if make_table:
                        tn = wp.tile([128, F], BF, name="tn", tag="tn")
                        nc.vector.tensor_tensor(
                            tn[:], fn[:], normb_t[:, ds(g, 1)].broadcast_to([128, F]),
                            mybir.AluOpType.mult)
                        nc.sync.dma_start(ag_in[ds(g * 128, 128), :], tn[:])
                    pt = ptr.tile([128, 128], BF, name="pt2", tag="tr")
                    nc.tensor.transpose(pt[:], fn[:], ident_b[:])
                    nc.vector.tensor_copy(fT[:, ds(g * 128, 128)], pt[:])

            for l in range(HLAYERS + 1) if not STOP else []:
                hop(T_in, f1T, make_table=True)
                if l == 0 and LVL <= ORDER.index("hop1"):
                    bail()
                    STOP = True
                    break
                nc.gpsimd.collective_compute(
                    "AllGather", mybir.AluOpType.bypass, replica_groups=RG,
                    ins=[ag_in.opt()], outs=[T_hop.opt()])
                if l == 0 and LVL <= ORDER.index("aghop"):
                    bail()
                    STOP = True
                    break
                hop(T_hop, f2T, make_table=False)
                if l == 0 and LVL <= ORDER.index("hop2"):
                    bail()
                    STOP = True
                    break
                fTs = [f0T, f1T, f2T]
                with tc.For_i(0, GRP, 1) as g:
                    ph = pmm.tile([128, 128], FP, name="ph", tag="mm")
                    for k in range(HOPS + 1):
                        nc.tensor.matmul(ph[:], w_t[l][:, k, :],
                                         fTs[k][:, ds(g * 128, 128)],
                                         start=(k == 0), stop=(k == HOPS))
                    act = wp.tile([128, 128], BF, name="act", tag="act")
                    nc.scalar.activation(act[:], ph[:],
                                         mybir.ActivationFunctionType.Relu,
                                         bias=misc_t[:, MC_B + l : MC_B + l + 1],
                                         scale=misc_t[:, MC_WS + l : MC_WS + l + 1])
                    nc.vector.tensor_copy(f0T[:, ds(g * 128, 128)], act[:])
                    pt = ptr.tile([128, 128], BF, name="pt3", tag="tr")
                    nc.tensor.transpose(pt[:], act[:], ident_b[:])
                    if l < HLAYERS:
                        tn = wp.tile([128, F], BF, name="tn2", tag="tn")
                        nc.vector.tensor_tensor(
                            tn[:], pt[:], normb_t[:, ds(g, 1)].broadcast_to([128, F]),
                            mybir.AluOpType.mult)
                        nc.sync.dma_start(ag_in[ds(g * 128, 128), :], tn[:])
                    else:
                        rr = wp.tile([128, F + 1], BF, name="rr", tag="rr")
                        nc.vector.tensor_copy(rr[:, 0:F], pt[:])
                        nc.vector.tensor_copy(rr[:, F : F + 1], ones_b[:])
                        og = wp.tile([128, 128], BF, name="og", tag="og")
                        nc.vector.tensor_tensor(
                            og[:], gslot_t[:, ds(g, 1)].broadcast_to([128, 128]),
                            iota_b[:], mybir.AluOpType.is_equal)
                        pr = pro.tile([128, F + 1], FP, name="pr", tag="ro")
                        nc.tensor.matmul(pr[:], og[:], rr[:], start=True, stop=True)
                        nc.vector.tensor_tensor(roacc_t[:], roacc_t[:], pr[:],
                                                mybir.AluOpType.add)
                if l < HLAYERS:
                    nc.gpsimd.collective_compute(
                        "AllGather", mybir.AluOpType.bypass, replica_groups=RG,
                        ins=[ag_in.opt()], outs=[T_in.opt()])
                if l == 0 and LVL <= ORDER.index("layer0"):
                    bail()
                    STOP = True
                    break

            # ---- readout: all-reduce partial sums, mean, classify ----
            if not STOP:
                nc.sync.dma_start(ar_in[:, :], roacc_t[:])
                nc.gpsimd.collective_compute(
                    "AllReduce", mybir.AluOpType.add, replica_groups=RG,
                    ins=[ar_in.opt()], outs=[ar_out.opt()])
                nc.sync.dma_start(ro2_t[:], ar_out[:, :])
                nc.vector.tensor_scalar_max(cnt_t[:], ro2_t[:, F : F + 1], 1.0)
                nc.vector.reciprocal(rcp_t[:], cnt_t[:])
                nc.vector.tensor_tensor(hg_t[:], ro2_t[:, 0:F],
                                        rcp_t[:].broadcast_to([128, F]),
                                        mybir.AluOpType.mult)
                ptf = ptr.tile([128, 128], FP, name="ptf", tag="tr")
                nc.tensor.transpose(ptf[:], hg_t[:], ident_f[:])
                nc.vector.tensor_copy(hgT_t[:], ptf[:])
                plog = pro.tile([128, F + 1], FP, name="plog", tag="ro")
                nc.tensor.matmul(plog[:, 0:CLASSES], hgT_t[:],
                                 misc_t[:, MC_WC : MC_WC + CLASSES],
                                 start=True, stop=True)
                nc.vector.tensor_tensor(logit_t[:], plog[:, 0:CLASSES],
                                        misc_t[:, MC_BC : MC_BC + CLASSES],
                                        mybir.AluOpType.add)
                nc.sync.dma_start(out_d[:, :], logit_t[:])

    nc.finalize()
    return nc


def _make_in_maps(x, graph_ids, Ws, bs, Wc, bc, idx_c, slot_cols):
    b_cols = np.stack(bs, axis=1).astype(np.float32)            # [128, 3]
    bc_rep = np.tile(np.asarray(bc, np.float32)[None, :], (128, 1))
    # permute W0's input rows to match the int5 unpack column order
    # (device column j*16+k holds original feature 8k+j), same perm in
    # each of the 3 hop blocks; W1/W2 consume unpermuted h -> untouched.
    Q = F // 8
    perm = np.array([8 * k + j for j in range(8) for k in range(Q)])
    W0p = np.asarray(Ws[0], np.float32).reshape(HOPS + 1, F, F)[:, perm, :]
    Ws = [W0p.reshape((HOPS + 1) * F, F)] + [np.asarray(w) for w in Ws[1:]]
    # int8 per-column quantization; dequant happens on the matmul output
    # via the activation's per-partition scale (out_f is the partition dim).
    w8s, ws_cols = [], []
    for w in Ws:
        w = np.asarray(w, np.float32)
        ws = np.maximum(np.abs(w).max(axis=0), 1e-30) / 127.0
        w8s.append(np.clip(np.round(w / ws[None, :]), -127, 127).astype(np.int8))
        ws_cols.append(ws)
    ws_cols = np.stack(ws_cols, axis=1).astype(np.float32)      # [128, 3]
    wc_f = np.asarray(Wc, np.float32)
    # per-node int5 offset-binary quantization of x, 8 values per 5 bytes
    xs_full = np.ones(N, np.float32)
    amax = np.abs(x).max(axis=1)
    nz = amax > 0
    xs_full[nz] = amax[nz] / 15.0
    q = (np.clip(np.round(x / xs_full[:, None]), -15, 15) + 16).astype(np.int64)
    qq = q.reshape(N, Q, 8)
    bits = np.zeros((N, Q), np.int64)
    for j in range(8):
        bits |= qq[:, :, j] << (5 * j)
    xbytes = np.stack([((bits >> (8 * i)) & 255) for i in range(5)],
                      axis=1).astype(np.uint8)                  # [N, 5, Q]
    # weights packed slot-major: [128, 9*128] int8
    w_pack = np.ascontiguousarray(
        np.concatenate(w8s, axis=0).reshape(3 * (HOPS + 1), 128, F)
        .transpose(1, 0, 2)
    ).reshape(128, -1)
    in_maps = []
    for c in range(NCORES):
        # pad rows decode to q=16 everywhere -> (16-16)*scale = 0
        pad_bits = 0
        for j in range(8):
            pad_bits |= 16 << (5 * j)
        x_loc = np.empty((NPAD, 5, Q), np.uint8)
        for i in range(5):
            x_loc[:, i] = (pad_bits >> (8 * i)) & 255
        x_loc[:PER] = xbytes[c * PER : (c + 1) * PER]
        x_pack = np.ascontiguousarray(
            x_loc.reshape(GRP, 128, XB).transpose(1, 0, 2)
        ).reshape(128, GRP * XB).view(np.int8)
        xs = np.ones(NPAD, np.float32)
        xs[:PER] = xs_full[c * PER : (c + 1) * PER]
        gsl = np.full(NPAD, 255.0, np.float32)
        gsl[:PER] = graph_ids[c * PER : (c + 1) * PER].astype(np.float32)
        misc = np.concatenate([
            b_cols, ws_cols, wc_f, bc_rep,
            np.ascontiguousarray(xs.reshape(GRP, 128).T),
            np.ascontiguousarray((-16.0 * xs).reshape(GRP, 128).T),
            np.ascontiguousarray(gsl.reshape(GRP, 128).T),
        ], axis=1).astype(np.float32)
        W16 = idx_c.shape[2]
        idx_pack = np.ascontiguousarray(
            idx_c[c].reshape(16, 8, W16 // 8).transpose(1, 0, 2)
        ).reshape(128, W16 // 8).view(np.int8)
        parts = [x_pack, slot_cols[c].view(np.int8), w_pack,
                 np.ascontiguousarray(misc).view(np.int8), idx_pack]
        X_OFF, SLOT_OFF, W_OFF, MISC_OFF, IDX_OFF, PCOLS = _pack_offsets(
            slot_cols.shape[2])
        pack = np.zeros((128, PCOLS), np.int8)
        for p, o in zip(parts, (X_OFF, SLOT_OFF, W_OFF, MISC_OFF, IDX_OFF)):
            pack[:, o : o + p.shape[1]] = p
        in_maps.append(dict(pack=pack))
    return in_maps


XB = F // 8 * 5              # packed int5 bytes per node (8 values / 5 bytes)


def _pack_offsets(NCH):
    MCOLS = 2 * (HLAYERS + 1) + CLASSES + CLASSES + GRP + GRP + GRP
    W16 = NCH * 128 // 16
    SLOT_OFF = GRP * XB
    W_OFF = -(-(SLOT_OFF + NCH) // 4) * 4
    MISC_OFF = -(-(W_OFF + (HLAYERS + 1) * (HOPS + 1) * F) // 4) * 4
    IDX_OFF = MISC_OFF + MCOLS * 4
    PCOLS = IDX_OFF + (W16 // 8) * 2
    return 0, SLOT_OFF, W_OFF, MISC_OFF, IDX_OFF, PCOLS


def kernel(x, src, dst, graph_ids, W0, b0, W1, b1, W2, b2, Wc, bc, **_):
    x = np.asarray(x, np.float32)
    graph_ids = np.asarray(graph_ids, np.int64)
    idx_c, slot_cols, CAu, CBu = _prep_edges(src, dst)
    nc = _build_program(CAu, CBu)
    in_maps = _make_in_maps(
        x, graph_ids,
        [np.asarray(W0), np.asarray(W1), np.asarray(W2)],
        [np.asarray(b0, np.float32), np.asarray(b1, np.float32),
         np.asarray(b2, np.float32)],
        Wc, bc, idx_c, slot_cols)
    last_err = None
    for _attempt in range(3):   # retry transient device wedges (NRT_* errors)
        try:
            res = bass_utils.run_bass_kernel_spmd(
                nc, in_maps, core_ids=list(range(NCORES)))
            return np.asarray(res.results[0]["out"], np.float32)
        except Exception as e:  # noqa: BLE001
            last_err = e
            try:
                jax.clear_backends()   # drop a wedged PJRT client
            except Exception:  # noqa: BLE001
                pass
    raise last_err
